# revision 20
# baseline (speedup 1.0000x reference)
"""Trainium2 Bass kernel for nn_ClassifierGuided (2-modality top-12-of-16 MoE classifier).

Sharding: pure data-parallel over tokens. 2 modalities x 4096 tokens = 8192
tokens; each of the 8 cores owns 1024 tokens of one modality (cores 0-3 ->
modality 0, cores 4-7 -> modality 1) and that modality's full weights.
Dense-eval MoE (all 16 experts computed, sparse gates applied), so no
all-to-all is needed.

Precision: expert MLP + b2 close in fp8 e4m3 via DoubleRow matmuls (two
128-deep contraction planes per instruction at 0.5 cycles/row = 4x the fp32r
rate). Gating, residual and head run in bf16; top-12 selection flips are rare
near-ties with negligible gate deltas. Measured end-to-end error ~4e-3
against the fp32 reference (tolerance 2e-2).

Layout: 24 h-chunks of 128. Chunks 0-15 are single-expert ("pure": expert e
keeps h[0:128] if e even else h[64:192]); chunks 16-23 are half/half mixed
(expert 2j h[128:192] on partitions 0:64, expert 2j+1 h[0:64] on 64:128).
Gates stream to DRAM as fp8 and come back as a per-chunk broadcast table
[128, 24, NT] in 3 strided DMAs, so the gate multiply is one Pool op per
chunk. Quad q = experts 4q..4q+3 = chunks [4q..4q+4) + [16+2q, 17+2q].

Pipeline: W2 DoubleRows of quad q-1 interleave with W1 chunks of quad q so
the in-order PE never stalls on the 2-bank h-PSUM rotation; relu+bias splits
between ACT (4/quad) and DVE (2/quad); tile 1's first quad runs inside tile
0's close; gating for the second token half is issued mid-pipeline.
"""
import sys

sys.path.insert(0, "/opt/trn_rl_repo")

import numpy as np
import ml_dtypes

import concourse.bass as bass
import concourse.mybir as mybir
import concourse.tile as tile
from concourse import bacc
from concourse.bass_utils import run_bass_kernel_spmd
from concourse.masks import make_identity

# ---- problem sizes (hardcoded per the harness contract) ----
B = 4096           # tokens per modality
D = 768            # model dim
E = 16             # experts
H = 192            # expert hidden
O = 101            # classifier out
KTOP = 12          # top-k experts
NCORES = 8
BC = B // 4        # 1024 tokens per core
DC = D // 128      # 6 d-chunks
NT = 512           # token tile (matmul moving dim / PSUM bank)
NTILES = BC // NT  # 2
NQ = 4             # expert quads
NCH = E * H // 128  # 24 h-chunks
F32 = mybir.dt.float32
BF16 = mybir.dt.bfloat16
F8 = mybir.dt.float8e4
DR = mybir.MatmulPerfMode.DoubleRow
NEG_BIG = -1.0e30
NPF8 = ml_dtypes.float8_e4m3
NPBF = ml_dtypes.bfloat16

_NC_CACHE = {}


def _hperm():
    """Global h-permutation: 16 pure chunks then 8 mixed chunks."""
    idx = []
    for e in range(E):
        lo = 0 if e % 2 == 0 else 64
        idx.extend(e * H + h for h in range(lo, lo + 128))
    for j in range(8):
        idx.extend((2 * j) * H + h for h in range(128, 192))
        idx.extend((2 * j + 1) * H + h for h in range(0, 64))
    return np.array(idx)


HPERM = _hperm()
# quad q covers chunks [4q, 4q+1, 4q+2, 4q+3, 16+2q, 17+2q]
QCHUNK = [[4 * q, 4 * q + 1, 4 * q + 2, 4 * q + 3, 16 + 2 * q, 17 + 2 * q]
          for q in range(NQ)]


def build_nc():
    nc = bacc.Bacc("TRN2", target_bir_lowering=False, debug=False,
                   num_devices=NCORES)

    # ---- DRAM I/O (per-core views; host pre-packs + pre-quantizes) ----
    xbf = nc.dram_tensor("xbf", [D, BC], BF16, kind="ExternalInput").ap()
    x8d = nc.dram_tensor("x8d", [D, BC], F8, kind="ExternalInput").ap()
    w1p = nc.dram_tensor("w1p", [D, E * H], F8, kind="ExternalInput").ap()
    w2p = nc.dram_tensor("w2p", [E * H, D], F8, kind="ExternalInput").ap()
    b1p = nc.dram_tensor("b1p", [128, NCH], F32, kind="ExternalInput").ap()
    b28 = nc.dram_tensor("b28", [8, 2 * D], F8, kind="ExternalInput").ap()
    wg = nc.dram_tensor("wg", [D, E], BF16, kind="ExternalInput").ap()
    wo = nc.dram_tensor("wo", [D, O], BF16, kind="ExternalInput").ap()
    bo = nc.dram_tensor("bo", [O, 1], F32, kind="ExternalInput").ap()
    outT = nc.dram_tensor("outT", [O, BC], F32, kind="ExternalOutput").ap()

    xv = xbf.rearrange("(c p) b -> p c b", p=128)
    x8v = x8d.rearrange("(c p) b -> p c b", p=128)
    w1v = w1p.rearrange("(c p) h -> p c h", p=128)
    w2v = w2p.rearrange("(k p) d -> p k d", p=128)
    wgv = wg.rearrange("(c p) e -> p c e", p=128)
    wov = wo.rearrange("(c p) o -> p c o", p=128)

    with tile.TileContext(nc) as tc:
        with tc.tile_pool(name="const", bufs=1) as cpool:
            xsb = cpool.tile([128, DC, BC], BF16)       # x, later z in place
            x8sb = cpool.tile([128, DC, BC], F8)
            w1sb = cpool.tile([128, DC, E * H], F8)
            w2sb = cpool.tile([128, NCH, D], F8)
            b1sb = cpool.tile([128, NCH], F32)
            b2sb = cpool.tile([8, 2, D], F8)
            wgsb = cpool.tile([128, DC, E], BF16)
            wosb = cpool.tile([128, DC, O], BF16)
            bosb = cpool.tile([O, 1], F32)
            zeros = cpool.tile([128, NT], F32)
            gdram = cpool.tile([E, BC], F8, space="DRAM")

            nc.vector.memset(zeros[:, :], 0.0)

            # ---- load schedule (SP + ACT hwdge queues; gb reads on both
            # SP and Pool). Order is critical: engines are in-order, and a
            # queued DMA blocks later compute on the same engine. ----
            def xq(i, eng):   # quarter of xbf (256 tokens, innermost 512B)
                eng.dma_start(out=xsb[:, :, 256 * i:256 * (i + 1)],
                              in_=xv[:, :, 256 * i:256 * (i + 1)])

            def load_w1q(q):
                HQ = 4 * H
                nc.sync.dma_start(out=w1sb[:, :, HQ * q:HQ * (q + 1)],
                                  in_=w1v[:, :, HQ * q:HQ * (q + 1)])

            def load_w2(k0, k1):   # W2 chunk-rows k0:k1
                nc.sync.dma_start(out=w2sb[:, k0:k1, :], in_=w2v[:, k0:k1, :])

            # SP queue carries every weight in deadline order; the ACT queue
            # stays clear so gating's exp ops run at ~3us.
            nc.sync.dma_start(out=wgsb[:, :, :], in_=wgv)
            xq(1, nc.sync)
            nc.sync.dma_start(out=x8sb[:, :, 0:NT], in_=x8v[:, :, 0:NT])
            load_w1q(0)
            nc.sync.dma_start(out=b1sb[:, :], in_=b1p)
            load_w1q(1)
            load_w2(0, 4)
            load_w2(4, 8)
            load_w1q(2)
            load_w2(16, 20)
            load_w1q(3)
            load_w2(8, 12)
            load_w2(20, 24)
            nc.sync.dma_start(out=x8sb[:, :, NT:], in_=x8v[:, :, NT:])
            load_w2(12, 16)
            nc.sync.dma_start(out=wosb[:, :, :], in_=wov)
            nc.sync.dma_start(out=b2sb[:, :, :],
                              in_=b28.rearrange("p (k d) -> p k d", k=2))
            nc.sync.dma_start(out=bosb[:, :], in_=bo)

            xq(0, nc.scalar)

            # gate-broadcast table reads: fp8 gates round-trip through DRAM,
            # partition-step-0 reads build gball [128, 24, NT]
            gb_tiles = {}
            ctx_gb = tc.tile_pool(name="gball", bufs=2)
            gbpool = ctx_gb.__enter__()

            def gb_alloc(t):
                gb_tiles[t] = gbpool.tile([128, NCH, NT], F8, tag="gb",
                                          name="gball")
                return gb_tiles[t]

            def gb_pure(t, q0, q1, eng):
                # pure chunk cols q0*4 : q1*4 (rows = experts, stride BC)
                gb = gb_tiles[t]
                eng.dma_start(
                    out=gb[:, 4 * q0:4 * q1, :],
                    in_=bass.AP(tensor=gdram.tensor,
                                offset=4 * q0 * BC + NT * t,
                                ap=[[0, 128], [BC, 4 * (q1 - q0)], [1, NT]]))

            def gb_mixed(t, eng):
                # mixed cols 16:24: even expert rows on partitions 0:64,
                # odd expert rows on partitions 64:128
                gb = gb_tiles[t]
                eng.dma_start(
                    out=gb[0:64, 16:24, :],
                    in_=bass.AP(tensor=gdram.tensor, offset=NT * t,
                                ap=[[0, 64], [2 * BC, 8], [1, NT]]))
                eng.dma_start(
                    out=gb[64:128, 16:24, :],
                    in_=bass.AP(tensor=gdram.tensor, offset=BC + NT * t,
                                ap=[[0, 64], [2 * BC, 8], [1, NT]]))

            # ---------------- gating (bf16 logits, exact-enough top-12) ----
            gate_ctxs = [tc.tile_pool(name="gsb", bufs=3)]
            gsb = gate_ctxs[0].__enter__()

            def gating_half(hf, hps):
                for i in range(4 * hf, 4 * hf + 4):
                    ts = slice(128 * i, 128 * (i + 1))
                    lg_ps = hps.tile([128, E], F32, tag="h", name="lg_ps")
                    for c in range(DC):
                        nc.tensor.matmul(lg_ps[:, :], xsb[:, c, ts],
                                         wgsb[:, c, :],
                                         start=(c == 0), stop=(c == DC - 1))
                    lg = gsb.tile([128, E], F32, tag="lg_sb")
                    nc.vector.tensor_copy(lg[:, :], lg_ps[:, :])
                    # exp on ACT runs concurrently with the DVE top-k chain
                    e16 = gsb.tile([128, E], F32, tag="e16")
                    nc.scalar.activation(e16[:, :], lg[:, :],
                                         mybir.ActivationFunctionType.Exp)
                    t8a = gsb.tile([128, 8], F32, tag="t8a")
                    nc.vector.max(t8a[:, :], lg[:, :])
                    l2 = gsb.tile([128, E], F32, tag="l2")
                    nc.vector.match_replace(l2[:, :], t8a[:, :], lg[:, :],
                                            NEG_BIG)
                    t8b = gsb.tile([128, 8], F32, tag="t8b")
                    nc.vector.max(t8b[:, :], l2[:, :])
                    em = gsb.tile([128, E], F32, tag="em")
                    ssum = gsb.tile([128, 1], F32, tag="ssum")
                    nc.vector.scalar_tensor_tensor(
                        out=em[:, :], in0=lg[:, :], scalar=t8b[:, 3:4],
                        in1=e16[:, :], op0=mybir.AluOpType.is_ge,
                        op1=mybir.AluOpType.mult, accum_out=ssum[:, :])
                    rinv = gsb.tile([128, 1], F32, tag="rinv")
                    nc.vector.reciprocal(rinv[:, :], ssum[:, :])
                    g = gsb.tile([128, E], F32, tag="g")
                    nc.vector.tensor_scalar_mul(g[:, :], em[:, :], rinv[:, :])
                    # transposing cast DMA: gates straight to the fp8 DRAM
                    # table (row = expert, col = token); replaces the PE
                    # transpose + copies + flush
                    nc.gpsimd.dma_start(
                        out=bass.AP(tensor=gdram.tensor, offset=128 * i,
                                    ap=[[1, 128], [BC, E]]),
                        in_=g[:, :])

            # ---------------- main pipeline ----------------
            with tc.tile_pool(name="moeps", bufs=DC, space="PSUM") as moeps, \
                 tc.tile_pool(name="hps", bufs=2, space="PSUM") as hps, \
                 tc.tile_pool(name="hsb", bufs=4) as hsbpool, \
                 tc.tile_pool(name="hg8", bufs=2) as hg8pool, \
                 tc.tile_pool(name="opool", bufs=2) as opool:

                hg_tiles = {}
                moe_tiles = {}
                out_ps_box = {}
                g8_tiles = {}

                def load_g8(t, eng):
                    # b2-close rhs: gates as [8, 2, NT] fp8 (e = p + 8*blk)
                    g8 = gsb.tile([8, 2, NT], F8, tag="g8", name="g8")
                    eng.dma_start(
                        out=g8[:, :, :],
                        in_=bass.AP(tensor=gdram.tensor, offset=NT * t,
                                    ap=[[BC, 8], [8 * BC, 2], [1, NT]]))
                    g8_tiles[t] = g8

                def w1_chunk(t, q, m):
                    # 3 W1 DoubleRows -> relu+bias (ACT or DVE) -> Pool gate
                    # multiply into hg[(t,q)][:, m, :] (fp8)
                    ts = slice(NT * t, NT * (t + 1))
                    k = QCHUNK[q][m]
                    hp = hps.tile([128, NT], F32, tag="h", name="h")
                    for c2 in range(3):
                        nc.tensor.matmul(hp[:, :],
                                         w1sb[:, 2 * c2:2 * c2 + 2,
                                              128 * k:128 * (k + 1)],
                                         x8sb[:, 2 * c2:2 * c2 + 2, ts],
                                         start=(c2 == 0), stop=(c2 == 2),
                                         perf_mode=DR)
                    hs_t = hsbpool.tile([128, NT], F32, tag="hs")
                    # 2 of 6 relus per quad go to DVE; the final quad of the
                    # final tile splits 3/3 so its last hg chunk (which gates
                    # the whole close) lands ~1.7us earlier
                    dve_m = (1, 3, 5) if (t, q) == (1, 3) else (1, 4)
                    if m in dve_m:
                        nc.vector.scalar_tensor_tensor(
                            out=hs_t[:, :], in0=hp[:, :],
                            scalar=b1sb[:, k:k + 1], in1=zeros[:, :],
                            op0=mybir.AluOpType.add, op1=mybir.AluOpType.max)
                    else:
                        nc.scalar.activation(hs_t[:, :], hp[:, :],
                                             mybir.ActivationFunctionType.Relu,
                                             bias=b1sb[:, k:k + 1])
                    nc.gpsimd.tensor_tensor(
                        out=hg_tiles[(t, q)][:, m, :], in0=hs_t[:, :],
                        in1=gb_tiles[t][:, k, :], op=mybir.AluOpType.mult)

                def w2_slot(t, q, m):
                    # 3 of quad q's 18 W2 DoubleRows (pair-major order)
                    moe = moe_tiles[t]
                    for idx in range(3 * m, 3 * m + 3):
                        j, c = divmod(idx, DC)
                        kp = 4 * q + 2 * j if j < 2 else 16 + 2 * q
                        nc.tensor.matmul(moe[c][:, :],
                                         w2sb[:, kp:kp + 2,
                                              128 * c:128 * (c + 1)],
                                         hg_tiles[(t, q)][:, 2 * j:2 * j + 2, :],
                                         start=(q == 0 and j == 0),
                                         stop=False, perf_mode=DR)

                def stage1(t, q, prev=None, post_m=None):
                    gb = gb_tiles.get(t)
                    hg_tiles[(t, q)] = hg8pool.tile([128, DC, NT], F8,
                                                    tag="hg", name="hg")
                    for m in range(DC):
                        w1_chunk(t, q, m)
                        if prev is not None:
                            w2_slot(t, prev[1], m)
                        if post_m is not None and m in post_m:
                            post_m[m]()

                def finish_chunk(t, c):
                    # z = relu(moe) + x in one DVE op (bf16 out, in place)
                    ts = slice(NT * t, NT * (t + 1))
                    moe = moe_tiles[t]
                    nc.vector.scalar_tensor_tensor(
                        out=xsb[:, c, ts], in0=moe[c][:, :], scalar=0.0,
                        in1=xsb[:, c, ts], op0=mybir.AluOpType.max,
                        op1=mybir.AluOpType.add)

                def head_chunk(t, c):
                    ts = slice(NT * t, NT * (t + 1))
                    if t not in out_ps_box:
                        out_ps_box[t] = moeps.tile([O, NT], F32, tag="moe",
                                                   name="out_ps")
                    nc.tensor.matmul(out_ps_box[t][:, :], wosb[:, c, :],
                                     xsb[:, c, ts],
                                     start=(c == 0), stop=(c == DC - 1))

                def close_tile(t, interleave_next=False):
                    ts = slice(NT * t, NT * (t + 1))
                    moe = moe_tiles[t]
                    # part A: final quad's j=0,1 DoubleRows (only need the
                    # quad's first 4 hg chunks) + next tile's first W1 quad
                    for c in range(DC):
                        if interleave_next and c == 0:
                            gb_alloc(t + 1)
                            gb_pure(t + 1, 0, 4, nc.sync)
                            gb_mixed(t + 1, nc.sync)
                            load_g8(t + 1, nc.sync)
                            hg_tiles[(t + 1, 0)] = hg8pool.tile(
                                [128, DC, NT], F8, tag="hg", name="hg")
                        for j in range(2):
                            nc.tensor.matmul(moe[c][:, :],
                                             w2sb[:, 12 + 2 * j:14 + 2 * j,
                                                  128 * c:128 * (c + 1)],
                                             hg_tiles[(t, 3)][:, 2 * j:2 * j + 2, :],
                                             start=False, stop=False,
                                             perf_mode=DR)
                        if interleave_next:
                            w1_chunk(t + 1, 0, c)
                    # part B: per-chunk mixed-pair close, b2 bias, residual
                    # drain, trailing head
                    for c in range(DC):
                        nc.tensor.matmul(moe[c][:, :],
                                         w2sb[:, 22:24, 128 * c:128 * (c + 1)],
                                         hg_tiles[(t, 3)][:, 4:6, :],
                                         start=False, stop=False, perf_mode=DR)
                        nc.tensor.matmul(moe[c][:, :],
                                         b2sb[:, :, 128 * c:128 * (c + 1)],
                                         g8_tiles[t][:, :, :],
                                         start=False, stop=True, perf_mode=DR)
                        finish_chunk(t, c)
                        if c >= 1:
                            head_chunk(t, c - 1)
                    head_chunk(t, DC - 1)
                    osb = opool.tile([O, NT], F32, tag="osb")
                    nc.scalar.activation(osb[:, :], out_ps_box[t][:, :],
                                         mybir.ActivationFunctionType.Identity,
                                         bias=bosb[:, :])
                    nc.sync.dma_start(out=outT[:, ts], in_=osb[:, :])

                # ---- driver ----
                gating_half(0, hps)
                gb_alloc(0)
                gb_pure(0, 0, 1, nc.gpsimd)
                gb_mixed(0, nc.gpsimd)
                load_g8(0, nc.gpsimd)

                moe_tiles[0] = [moeps.tile([128, NT], F32, tag="moe",
                                           name="moe") for _ in range(DC)]
                stage1(0, 0,
                       post_m={1: lambda: xq(2, nc.scalar),
                               5: lambda: gb_pure(0, 1, 2, nc.gpsimd)})
                stage1(0, 1, prev=(0, 0),
                       post_m={1: lambda: xq(3, nc.scalar),
                               5: lambda: gb_pure(0, 2, 3, nc.gpsimd)})
                stage1(0, 2, prev=(0, 1),
                       post_m={5: lambda: gb_pure(0, 3, 4, nc.gpsimd)})
                # second-half gating issued mid-pipeline (engines in-order;
                # its PE/ACT/DVE slices fit the per-quad slack here)
                gating_half(1, hps)
                stage1(0, 3, prev=(0, 2))
                close_tile(0, interleave_next=True)

                moe_tiles[1] = [moeps.tile([128, NT], F32, tag="moe",
                                           name="moe") for _ in range(DC)]
                stage1(1, 1, prev=(1, 0))
                stage1(1, 2, prev=(1, 1))
                stage1(1, 3, prev=(1, 2))
                close_tile(1)

            gate_ctxs[0].__exit__(None, None, None)
            ctx_gb.__exit__(None, None, None)

    nc.compile()
    return nc


def _pack_core_inputs(x, Wg, W1, b1, W2, b2, Wo, bo, c4):
    """Per-core input dict for one modality's weights + 1024-token slice."""
    f = np.float32
    tok = slice(BC * c4, BC * (c4 + 1))
    xt = np.ascontiguousarray(np.asarray(x[tok], f).T)
    w1f = np.asarray(W1, f).transpose(1, 0, 2).reshape(D, E * H)[:, HPERM]
    w2f = np.asarray(W2, f).reshape(E * H, D)[HPERM, :]
    b1f = np.asarray(b1, f).reshape(E * H)[HPERM]
    b2f = np.asarray(b2, f)          # [16, D]; row e -> [e % 8, (e//8)*D]
    b28 = np.concatenate([b2f[0:8], b2f[8:16]], axis=1)
    return {
        "xbf": xt.astype(NPBF),
        "x8d": xt.astype(NPF8),
        "w1p": np.ascontiguousarray(w1f.astype(NPF8)),
        "w2p": np.ascontiguousarray(w2f.astype(NPF8)),
        "b1p": np.ascontiguousarray(b1f.reshape(NCH, 128).T),
        "b28": np.ascontiguousarray(b28.astype(NPF8)),
        "wg": np.ascontiguousarray(np.asarray(Wg, f).astype(NPBF)),
        "wo": np.ascontiguousarray(np.asarray(Wo, f).astype(NPBF)),
        "bo": np.ascontiguousarray(np.asarray(bo, f).reshape(O, 1)),
    }


def run_on_hw(inputs, trace=False, **kw):
    if "nc" not in _NC_CACHE:
        _NC_CACHE["nc"] = build_nc()
    nc = _NC_CACHE["nc"]
    in_maps = []
    for core in range(NCORES):
        i, c4 = divmod(core, 4)
        x = inputs["x0"] if i == 0 else inputs["x1"]
        in_maps.append(_pack_core_inputs(
            x, inputs["Wg"][i], inputs["W1"][i], inputs["b1"][i],
            inputs["W2"][i], inputs["b2"][i], inputs["Wo"][i], inputs["bo"][i], c4))
    res = run_bass_kernel_spmd(nc, in_maps, core_ids=list(range(NCORES)),
                               trace=trace, **kw)
    outs = []
    for i in range(2):
        outs.append(np.concatenate(
            [res.results[4 * i + c]["outT"].T for c in range(4)], axis=0))
    return (outs[0], outs[1]), res


def kernel(**inputs):
    (o0, o1), _ = run_on_hw(inputs)
    return (o0, o1)


# revision 21
# speedup vs baseline: 1.0435x; 1.0435x over previous
"""Trainium2 Bass kernel for nn_ClassifierGuided (2-modality top-12-of-16 MoE classifier).

Sharding: pure data-parallel over tokens. 2 modalities x 4096 tokens = 8192
tokens; each of the 8 cores owns 1024 tokens of one modality (cores 0-3 ->
modality 0, cores 4-7 -> modality 1) and that modality's full weights.
Dense-eval MoE (all 16 experts computed, sparse gates applied), so no
all-to-all is needed.

Precision: expert MLP + b2 close in fp8 e4m3 via DoubleRow matmuls (two
128-deep contraction planes per instruction at 0.5 cycles/row = 4x the fp32r
rate). Gating, residual and head run in bf16; top-12 selection flips are rare
near-ties with negligible gate deltas. Measured end-to-end error ~4e-3
against the fp32 reference (tolerance 2e-2).

Layout: 24 h-chunks of 128. Chunks 0-15 are single-expert ("pure": expert e
keeps h[0:128] if e even else h[64:192]); chunks 16-23 are half/half mixed
(expert 2j h[128:192] on partitions 0:64, expert 2j+1 h[0:64] on 64:128).
Gates stream to DRAM as fp8 and come back as a per-chunk broadcast table
[128, 24, NT] in 3 strided DMAs, so the gate multiply is one Pool op per
chunk. Quad q = experts 4q..4q+3 = chunks [4q..4q+4) + [16+2q, 17+2q].

Pipeline: W2 DoubleRows of quad q-1 interleave with W1 chunks of quad q so
the in-order PE never stalls on the 2-bank h-PSUM rotation; relu+bias splits
between ACT (4/quad) and DVE (2/quad); tile 1's first quad runs inside tile
0's close; gating for the second token half is issued mid-pipeline.
"""
import sys

sys.path.insert(0, "/opt/trn_rl_repo")

import numpy as np
import ml_dtypes

import concourse.bass as bass
import concourse.mybir as mybir
import concourse.tile as tile
from concourse import bacc
from concourse.bass_utils import run_bass_kernel_spmd
from concourse.masks import make_identity

# ---- problem sizes (hardcoded per the harness contract) ----
B = 4096           # tokens per modality
D = 768            # model dim
E = 16             # experts
H = 192            # expert hidden
O = 101            # classifier out
KTOP = 12          # top-k experts
NCORES = 8
BC = B // 4        # 1024 tokens per core
DC = D // 128      # 6 d-chunks
NT = 512           # token tile (matmul moving dim / PSUM bank)
NTILES = BC // NT  # 2
NQ = 4             # expert quads
NCH = E * H // 128  # 24 h-chunks
F32 = mybir.dt.float32
BF16 = mybir.dt.bfloat16
F8 = mybir.dt.float8e4
DR = mybir.MatmulPerfMode.DoubleRow
NEG_BIG = -1.0e30
NPF8 = ml_dtypes.float8_e4m3
NPBF = ml_dtypes.bfloat16

_NC_CACHE = {}


def _hperm():
    """Global h-permutation: 16 pure chunks then 8 mixed chunks."""
    idx = []
    for e in range(E):
        lo = 0 if e % 2 == 0 else 64
        idx.extend(e * H + h for h in range(lo, lo + 128))
    for j in range(8):
        idx.extend((2 * j) * H + h for h in range(128, 192))
        idx.extend((2 * j + 1) * H + h for h in range(0, 64))
    return np.array(idx)


HPERM = _hperm()
# quad q covers chunks [4q, 4q+1, 4q+2, 4q+3, 16+2q, 17+2q]
QCHUNK = [[4 * q, 4 * q + 1, 4 * q + 2, 4 * q + 3, 16 + 2 * q, 17 + 2 * q]
          for q in range(NQ)]


def build_nc():
    nc = bacc.Bacc("TRN2", target_bir_lowering=False, debug=False,
                   num_devices=NCORES)

    # ---- DRAM I/O (per-core views; host pre-packs + pre-quantizes) ----
    xbf = nc.dram_tensor("xbf", [D, BC], BF16, kind="ExternalInput").ap()
    x8d = nc.dram_tensor("x8d", [D, BC], F8, kind="ExternalInput").ap()
    w1p = nc.dram_tensor("w1p", [D, E * H], F8, kind="ExternalInput").ap()
    w2p = nc.dram_tensor("w2p", [E * H, D], F8, kind="ExternalInput").ap()
    b1p = nc.dram_tensor("b1p", [128, NCH], F32, kind="ExternalInput").ap()
    b28 = nc.dram_tensor("b28", [8, 2 * D], F8, kind="ExternalInput").ap()
    wg = nc.dram_tensor("wg", [D, E], BF16, kind="ExternalInput").ap()
    wo = nc.dram_tensor("wo", [D, O], BF16, kind="ExternalInput").ap()
    bo = nc.dram_tensor("bo", [O, 1], F32, kind="ExternalInput").ap()
    outT = nc.dram_tensor("outT", [O, BC], F32, kind="ExternalOutput").ap()

    xv = xbf.rearrange("(c p) b -> p c b", p=128)
    x8v = x8d.rearrange("(c p) b -> p c b", p=128)
    w1v = w1p.rearrange("(c p) h -> p c h", p=128)
    w2v = w2p.rearrange("(k p) d -> p k d", p=128)
    wgv = wg.rearrange("(c p) e -> p c e", p=128)
    wov = wo.rearrange("(c p) o -> p c o", p=128)

    with tile.TileContext(nc) as tc:
        with tc.tile_pool(name="const", bufs=1) as cpool:
            xsb = cpool.tile([128, DC, BC], BF16)       # x, later z in place
            x8sb = cpool.tile([128, DC, BC], F8)
            w1sb = cpool.tile([128, DC, E * H], F8)
            w2sb = cpool.tile([128, NCH, D], F8)
            b1sb = cpool.tile([128, NCH], F32)
            b2sb = cpool.tile([8, 2, D], F8)
            wgsb = cpool.tile([128, DC, E], BF16)
            wosb = cpool.tile([128, DC, O], BF16)
            bosb = cpool.tile([O, 1], F32)
            zeros = cpool.tile([128, NT], F32)
            gdram = cpool.tile([E, BC], F8, space="DRAM")

            nc.vector.memset(zeros[:, :], 0.0)

            # ---- load schedule (SP + ACT hwdge queues; gb reads on both
            # SP and Pool). Order is critical: engines are in-order, and a
            # queued DMA blocks later compute on the same engine. ----
            def xq(i, eng):   # quarter of xbf (256 tokens, innermost 512B)
                eng.dma_start(out=xsb[:, :, 256 * i:256 * (i + 1)],
                              in_=xv[:, :, 256 * i:256 * (i + 1)])

            def load_w1q(q):
                HQ = 4 * H
                nc.sync.dma_start(out=w1sb[:, :, HQ * q:HQ * (q + 1)],
                                  in_=w1v[:, :, HQ * q:HQ * (q + 1)])

            def load_w2(k0, k1):   # W2 chunk-rows k0:k1
                nc.sync.dma_start(out=w2sb[:, k0:k1, :], in_=w2v[:, k0:k1, :])

            # SP queue carries every weight in deadline order; the ACT queue
            # stays clear so gating's exp ops run at ~3us.
            nc.sync.dma_start(out=wgsb[:, :, :], in_=wgv)
            xq(1, nc.sync)
            nc.sync.dma_start(out=x8sb[:, :, 0:NT], in_=x8v[:, :, 0:NT])
            load_w1q(0)
            nc.sync.dma_start(out=b1sb[:, :], in_=b1p)
            load_w1q(1)
            load_w2(0, 4)
            load_w2(4, 8)
            load_w1q(2)
            load_w2(16, 20)
            load_w1q(3)
            load_w2(8, 12)
            load_w2(20, 24)
            nc.sync.dma_start(out=x8sb[:, :, NT:], in_=x8v[:, :, NT:])
            load_w2(12, 16)
            nc.sync.dma_start(out=wosb[:, :, :], in_=wov)
            nc.sync.dma_start(out=b2sb[:, :, :],
                              in_=b28.rearrange("p (k d) -> p k d", k=2))
            nc.sync.dma_start(out=bosb[:, :], in_=bo)

            xq(0, nc.scalar)

            # gate-broadcast table reads: fp8 gates round-trip through DRAM,
            # partition-step-0 reads build gball [128, 24, NT]
            gb_tiles = {}
            ctx_gb = tc.tile_pool(name="gball", bufs=2)
            gbpool = ctx_gb.__enter__()

            def gb_alloc(t):
                gb_tiles[t] = gbpool.tile([128, NCH, NT], F8, tag="gb",
                                          name="gball")
                return gb_tiles[t]

            def gb_pure(t, q0, q1, eng):
                # pure chunk cols q0*4 : q1*4 (rows = experts, stride BC)
                gb = gb_tiles[t]
                eng.dma_start(
                    out=gb[:, 4 * q0:4 * q1, :],
                    in_=bass.AP(tensor=gdram.tensor,
                                offset=4 * q0 * BC + NT * t,
                                ap=[[0, 128], [BC, 4 * (q1 - q0)], [1, NT]]))

            def gb_mixed(t, eng):
                # mixed cols 16:24: even expert rows on partitions 0:64,
                # odd expert rows on partitions 64:128
                gb = gb_tiles[t]
                eng.dma_start(
                    out=gb[0:64, 16:24, :],
                    in_=bass.AP(tensor=gdram.tensor, offset=NT * t,
                                ap=[[0, 64], [2 * BC, 8], [1, NT]]))
                eng.dma_start(
                    out=gb[64:128, 16:24, :],
                    in_=bass.AP(tensor=gdram.tensor, offset=BC + NT * t,
                                ap=[[0, 64], [2 * BC, 8], [1, NT]]))

            # ---------------- gating (bf16 logits, exact-enough top-12) ----
            gate_ctxs = [tc.tile_pool(name="gsb", bufs=3)]
            gsb = gate_ctxs[0].__enter__()

            def gating_half(hf, hps):
                for i in range(4 * hf, 4 * hf + 4):
                    ts = slice(128 * i, 128 * (i + 1))
                    lg_ps = hps.tile([128, E], F32, tag="h", name="lg_ps")
                    for c in range(DC):
                        nc.tensor.matmul(lg_ps[:, :], xsb[:, c, ts],
                                         wgsb[:, c, :],
                                         start=(c == 0), stop=(c == DC - 1))
                    lg = gsb.tile([128, E], F32, tag="lg_sb")
                    nc.vector.tensor_copy(lg[:, :], lg_ps[:, :])
                    # exp on ACT runs concurrently with the DVE top-k chain
                    e16 = gsb.tile([128, E], F32, tag="e16")
                    nc.scalar.activation(e16[:, :], lg[:, :],
                                         mybir.ActivationFunctionType.Exp)
                    t8a = gsb.tile([128, 8], F32, tag="t8a")
                    nc.vector.max(t8a[:, :], lg[:, :])
                    l2 = gsb.tile([128, E], F32, tag="l2")
                    nc.vector.match_replace(l2[:, :], t8a[:, :], lg[:, :],
                                            NEG_BIG)
                    t8b = gsb.tile([128, 8], F32, tag="t8b")
                    nc.vector.max(t8b[:, :], l2[:, :])
                    em = gsb.tile([128, E], F32, tag="em")
                    ssum = gsb.tile([128, 1], F32, tag="ssum")
                    nc.vector.scalar_tensor_tensor(
                        out=em[:, :], in0=lg[:, :], scalar=t8b[:, 3:4],
                        in1=e16[:, :], op0=mybir.AluOpType.is_ge,
                        op1=mybir.AluOpType.mult, accum_out=ssum[:, :])
                    rinv = gsb.tile([128, 1], F32, tag="rinv")
                    nc.vector.reciprocal(rinv[:, :], ssum[:, :])
                    g = gsb.tile([128, E], F32, tag="g")
                    nc.vector.tensor_scalar_mul(g[:, :], em[:, :], rinv[:, :])
                    # transposing cast DMA: gates straight to the fp8 DRAM
                    # table (row = expert, col = token); replaces the PE
                    # transpose + copies + flush
                    nc.gpsimd.dma_start(
                        out=bass.AP(tensor=gdram.tensor, offset=128 * i,
                                    ap=[[1, 128], [BC, E]]),
                        in_=g[:, :])

            # ---------------- main pipeline ----------------
            with tc.tile_pool(name="moeps", bufs=DC, space="PSUM") as moeps, \
                 tc.tile_pool(name="hps", bufs=2, space="PSUM") as hps, \
                 tc.tile_pool(name="hsb", bufs=4) as hsbpool, \
                 tc.tile_pool(name="hg8", bufs=2) as hg8pool, \
                 tc.tile_pool(name="opool", bufs=2) as opool:

                hg_tiles = {}
                moe_tiles = {}
                out_ps_box = {}
                g8_tiles = {}

                def load_g8(t, eng):
                    # b2-close rhs: gates as [8, 2, NT] fp8 (e = p + 8*blk)
                    g8 = gsb.tile([8, 2, NT], F8, tag="g8", name="g8")
                    eng.dma_start(
                        out=g8[:, :, :],
                        in_=bass.AP(tensor=gdram.tensor, offset=NT * t,
                                    ap=[[BC, 8], [8 * BC, 2], [1, NT]]))
                    g8_tiles[t] = g8

                def w1_chunk(t, q, m):
                    # 3 W1 DoubleRows -> relu+bias (ACT or DVE) -> Pool gate
                    # multiply into hg[(t,q)][:, m, :] (fp8)
                    ts = slice(NT * t, NT * (t + 1))
                    k = QCHUNK[q][m]
                    hp = hps.tile([128, NT], F32, tag="h", name="h")
                    for c2 in range(3):
                        nc.tensor.matmul(hp[:, :],
                                         w1sb[:, 2 * c2:2 * c2 + 2,
                                              128 * k:128 * (k + 1)],
                                         x8sb[:, 2 * c2:2 * c2 + 2, ts],
                                         start=(c2 == 0), stop=(c2 == 2),
                                         perf_mode=DR)
                    hs_t = hsbpool.tile([128, NT], F32, tag="hs")
                    # 2 of 6 relus per quad go to DVE; the final quad of the
                    # final tile splits 3/3 so its last hg chunk (which gates
                    # the whole close) lands ~1.7us earlier
                    dve_m = (1, 3, 5) if (t, q) == (1, 3) else (1, 4)
                    if m in dve_m:
                        nc.vector.scalar_tensor_tensor(
                            out=hs_t[:, :], in0=hp[:, :],
                            scalar=b1sb[:, k:k + 1], in1=zeros[:, :],
                            op0=mybir.AluOpType.add, op1=mybir.AluOpType.max)
                    else:
                        nc.scalar.activation(hs_t[:, :], hp[:, :],
                                             mybir.ActivationFunctionType.Relu,
                                             bias=b1sb[:, k:k + 1])
                    nc.gpsimd.tensor_tensor(
                        out=hg_tiles[(t, q)][:, m, :], in0=hs_t[:, :],
                        in1=gb_tiles[t][:, k, :], op=mybir.AluOpType.mult)

                def w2_slot(t, q, m):
                    # 3 of quad q's 18 W2 DoubleRows (pair-major order)
                    moe = moe_tiles[t]
                    for idx in range(3 * m, 3 * m + 3):
                        j, c = divmod(idx, DC)
                        kp = 4 * q + 2 * j if j < 2 else 16 + 2 * q
                        nc.tensor.matmul(moe[c][:, :],
                                         w2sb[:, kp:kp + 2,
                                              128 * c:128 * (c + 1)],
                                         hg_tiles[(t, q)][:, 2 * j:2 * j + 2, :],
                                         start=(q == 0 and j == 0),
                                         stop=False, perf_mode=DR)

                def stage1(t, q, prev=None, post_m=None):
                    gb = gb_tiles.get(t)
                    hg_tiles[(t, q)] = hg8pool.tile([128, DC, NT], F8,
                                                    tag="hg", name="hg")
                    for m in range(DC):
                        w1_chunk(t, q, m)
                        if prev is not None:
                            w2_slot(t, prev[1], m)
                        if post_m is not None and m in post_m:
                            post_m[m]()

                def finish_chunk(t, c):
                    # z = relu(moe) + x in one DVE op (bf16 out, in place)
                    ts = slice(NT * t, NT * (t + 1))
                    moe = moe_tiles[t]
                    nc.vector.scalar_tensor_tensor(
                        out=xsb[:, c, ts], in0=moe[c][:, :], scalar=0.0,
                        in1=xsb[:, c, ts], op0=mybir.AluOpType.max,
                        op1=mybir.AluOpType.add)

                def head_chunk(t, c):
                    ts = slice(NT * t, NT * (t + 1))
                    if t not in out_ps_box:
                        out_ps_box[t] = moeps.tile([O, NT], F32, tag="moe",
                                                   name="out_ps")
                    nc.tensor.matmul(out_ps_box[t][:, :], wosb[:, c, :],
                                     xsb[:, c, ts],
                                     start=(c == 0), stop=(c == DC - 1))

                def close_tile(t, interleave_next=False):
                    ts = slice(NT * t, NT * (t + 1))
                    moe = moe_tiles[t]
                    # part A: final quad's j=0,1 DoubleRows (only need the
                    # quad's first 4 hg chunks) + next tile's first W1 quad
                    for c in range(DC):
                        if interleave_next and c == 0:
                            gb_alloc(t + 1)
                            gb_pure(t + 1, 0, 4, nc.sync)
                            gb_mixed(t + 1, nc.sync)
                            load_g8(t + 1, nc.sync)
                            hg_tiles[(t + 1, 0)] = hg8pool.tile(
                                [128, DC, NT], F8, tag="hg", name="hg")
                        for j in range(2):
                            nc.tensor.matmul(moe[c][:, :],
                                             w2sb[:, 12 + 2 * j:14 + 2 * j,
                                                  128 * c:128 * (c + 1)],
                                             hg_tiles[(t, 3)][:, 2 * j:2 * j + 2, :],
                                             start=False, stop=False,
                                             perf_mode=DR)
                        if interleave_next:
                            w1_chunk(t + 1, 0, c)
                    # part B: per-chunk mixed-pair close, b2 bias, residual
                    # drain, trailing head
                    for c in range(DC):
                        nc.tensor.matmul(moe[c][:, :],
                                         w2sb[:, 22:24, 128 * c:128 * (c + 1)],
                                         hg_tiles[(t, 3)][:, 4:6, :],
                                         start=False, stop=False, perf_mode=DR)
                        nc.tensor.matmul(moe[c][:, :],
                                         b2sb[:, :, 128 * c:128 * (c + 1)],
                                         g8_tiles[t][:, :, :],
                                         start=False, stop=True, perf_mode=DR)
                        finish_chunk(t, c)
                        if c >= 1:
                            head_chunk(t, c - 1)
                    head_chunk(t, DC - 1)
                    osb = opool.tile([O, NT], F32, tag="osb")
                    nc.scalar.activation(osb[:, :], out_ps_box[t][:, :],
                                         mybir.ActivationFunctionType.Identity,
                                         bias=bosb[:, :])
                    nc.sync.dma_start(out=outT[:, ts], in_=osb[:, :])

                # ---- driver ----
                gating_half(0, hps)
                gb_alloc(0)
                gb_pure(0, 0, 1, nc.gpsimd)
                gb_mixed(0, nc.gpsimd)
                load_g8(0, nc.gpsimd)

                moe_tiles[0] = [moeps.tile([128, NT], F32, tag="moe",
                                           name="moe") for _ in range(DC)]
                stage1(0, 0,
                       post_m={1: lambda: xq(2, nc.scalar),
                               5: lambda: gb_pure(0, 1, 2, nc.gpsimd)})
                stage1(0, 1, prev=(0, 0),
                       post_m={1: lambda: xq(3, nc.scalar),
                               5: lambda: gb_pure(0, 2, 3, nc.gpsimd)})
                # second-half gating issued mid-pipeline (engines in-order;
                # its PE/ACT/DVE slices fit the per-quad slack here)
                gating_half(1, hps)
                stage1(0, 2, prev=(0, 1),
                       post_m={5: lambda: gb_pure(0, 3, 4, nc.gpsimd)})
                stage1(0, 3, prev=(0, 2))
                close_tile(0, interleave_next=True)

                moe_tiles[1] = [moeps.tile([128, NT], F32, tag="moe",
                                           name="moe") for _ in range(DC)]
                stage1(1, 1, prev=(1, 0))
                stage1(1, 2, prev=(1, 1))
                stage1(1, 3, prev=(1, 2))
                close_tile(1)

            gate_ctxs[0].__exit__(None, None, None)
            ctx_gb.__exit__(None, None, None)

    nc.compile()
    return nc


def _pack_core_inputs(x, Wg, W1, b1, W2, b2, Wo, bo, c4):
    """Per-core input dict for one modality's weights + 1024-token slice."""
    f = np.float32
    tok = slice(BC * c4, BC * (c4 + 1))
    xt = np.ascontiguousarray(np.asarray(x[tok], f).T)
    w1f = np.asarray(W1, f).transpose(1, 0, 2).reshape(D, E * H)[:, HPERM]
    w2f = np.asarray(W2, f).reshape(E * H, D)[HPERM, :]
    b1f = np.asarray(b1, f).reshape(E * H)[HPERM]
    b2f = np.asarray(b2, f)          # [16, D]; row e -> [e % 8, (e//8)*D]
    b28 = np.concatenate([b2f[0:8], b2f[8:16]], axis=1)
    return {
        "xbf": xt.astype(NPBF),
        "x8d": xt.astype(NPF8),
        "w1p": np.ascontiguousarray(w1f.astype(NPF8)),
        "w2p": np.ascontiguousarray(w2f.astype(NPF8)),
        "b1p": np.ascontiguousarray(b1f.reshape(NCH, 128).T),
        "b28": np.ascontiguousarray(b28.astype(NPF8)),
        "wg": np.ascontiguousarray(np.asarray(Wg, f).astype(NPBF)),
        "wo": np.ascontiguousarray(np.asarray(Wo, f).astype(NPBF)),
        "bo": np.ascontiguousarray(np.asarray(bo, f).reshape(O, 1)),
    }


def run_on_hw(inputs, trace=False, **kw):
    if "nc" not in _NC_CACHE:
        _NC_CACHE["nc"] = build_nc()
    nc = _NC_CACHE["nc"]
    in_maps = []
    for core in range(NCORES):
        i, c4 = divmod(core, 4)
        x = inputs["x0"] if i == 0 else inputs["x1"]
        in_maps.append(_pack_core_inputs(
            x, inputs["Wg"][i], inputs["W1"][i], inputs["b1"][i],
            inputs["W2"][i], inputs["b2"][i], inputs["Wo"][i], inputs["bo"][i], c4))
    res = run_bass_kernel_spmd(nc, in_maps, core_ids=list(range(NCORES)),
                               trace=trace, **kw)
    outs = []
    for i in range(2):
        outs.append(np.concatenate(
            [res.results[4 * i + c]["outT"].T for c in range(4)], axis=0))
    return (outs[0], outs[1]), res


def kernel(**inputs):
    (o0, o1), _ = run_on_hw(inputs)
    return (o0, o1)
